# revision 25
# baseline (speedup 1.0000x reference)
"""Trainium2 Bass kernel for the DiscreteAutoregressiveFlow sampling problem.

Reference semantics (B=1024, L=1024, V=20, D=128):
    scan over t:  net = state @ W_out + b          [B, 2V]
                  m = argmax(net[:, :V]); s = argmax(net[:, V:])
                  u = ((a_t - m) * inv(s)) % V     (a_t = index of one-hot x_t,
                                                    inv(s) = mult. inverse mod V,
                                                    0 if s not coprime with V)
                  out_t = one_hot(u); state += emb[u]
Outputs ys[b, t] = one_hot(u_t).

Key structural property (exploited, then verified post-hoc): net_t depends on
the history only through the COUNT vector c_t of previously emitted symbols
(net_t = b + c_t @ (emb @ W_out)), and the dynamics have a self-reinforcing
attractor: the argmax pair (m_t, s_t) stops changing after t=2 and its margin
grows ~linearly in t (min margin 3.6 at t=32, 119 at t=1023 on the reference
input distribution). Hence for t >= 2 the scan collapses to the elementwise
map  u_t = ((a_t - m*) * p*) mod V  with per-row frozen (m*, p*).

Device kernel (pure data-parallel streaming, per core 128 batch rows):
    a5   = 40 - a_t              (weighted max over the one-hot x chunk)
    tp2  = (a5 - (20 - m*)) * p* = (20 + m* - a_t) * p*   in [0, 741]
    q    = floor(tp2 / 20)       (fp32 2^23 round-to-nearest trick, exact —
                                  constants validated exhaustively)
    oh   = one_hot(tp2 - 20q)    (is_eq against iota, written to ys)
The host computes the per-row frozen (m*, p*) by a 3-step bootstrap (tiny,
O(B*V) numpy - same spirit as the host-built emb @ W_out table), patches the
first 3 output steps, and then VERIFIES the full trajectory in vectorized
numpy: given the device output u, the recurrence check
    argmax(b + exclusive-cumsum(onehot(u)) @ EW)  ->  u
is embarrassingly parallel, and any self-consistent trajectory is (by
induction over t) THE unique reference trajectory. On verification failure
the slow-but-general sequential module (previous baseline, kept verbatim
below) is built and used instead.
"""

import numpy as np

B, L, V, D = 1024, 1024, 20, 128
NCORES = 8
BLOC = B // NCORES  # 128 batch rows per core
J2 = 2 * V          # 40

_CACHE = {}

# ---------------------------------------------------------------------------
# fast streaming module
# ---------------------------------------------------------------------------

# consts blob [128, FCW] column layout
_FOFF = {'wrow': 0, 'iota20m': 20, 'rr0': 40, 'pp': 41, 'ppfk': 42, 'cadd': 43}
FCW = 44

# floor((tp2)/20) via round-to-nearest: q1 = RN(tp2*FK + FC) = floor(tp2/20)+1
# for integer tp2 in [0, 741]; validated exhaustively (FC in [0.505, 0.547]).
FK = 3277.0 / 65536.0
FC = 0.525
FM = 8388608.0  # 2^23

TCH = 80                    # steps per chunk (DMA/compute overlap sweet spot)
TPATCH = 3                  # host-patched leading steps
XM_BF16 = True              # bf16 intermediate for the weighted-max pass
MULT_ENGINE = "gpsimd"      # engine for the x*w pass: "gpsimd" | "vector"


def _inv_table():
    inv = np.zeros(V, dtype=np.int64)
    for k in range(1, V):
        if np.gcd(k, V) == 1:
            inv[k] = pow(k, -1, V)
    return inv


def _chunk_schedule(nsteps, tch, ramp, tail=0):
    """Chunk sizes: optionally smaller leading/trailing chunks to shorten the
    pipeline fill/drain, full-width chunks in the middle."""
    sizes = []
    left = nsteps
    if ramp and nsteps > tch:
        r = min(ramp, left)
        sizes.append(r)
        left -= r
    tail = min(tail, left) if nsteps > tch else 0
    left -= tail
    while left > 0:
        s = min(tch, left)
        sizes.append(s)
        left -= s
    if tail:
        sizes.append(tail)
    return sizes


def _build_fast_module(nsteps, tch=None, xm_bf16=None, mult_engine=None,
                       xbufs=6, wbufs=4, obufs=6, ramp=32, tail=24,
                       barrier=False):
    import concourse.bacc as bacc
    import concourse.mybir as mybir
    import concourse.tile as tile

    if tch is None:
        tch = TCH
    if xm_bf16 is None:
        xm_bf16 = XM_BF16
    if mult_engine is None:
        mult_engine = MULT_ENGINE
    f32 = mybir.dt.float32
    xm_dt = mybir.dt.bfloat16 if xm_bf16 else f32
    nc = bacc.Bacc()

    x_d = nc.declare_dram_parameter("xloc", [BLOC, nsteps, V], f32, isOutput=False)
    consts_d = nc.declare_dram_parameter("consts", [128, FCW], f32, isOutput=False)
    ys_d = nc.declare_dram_parameter("ys", [BLOC, nsteps, V], f32, isOutput=True)

    tch = min(tch, nsteps)
    sched = _chunk_schedule(nsteps, tch, ramp, tail)

    sub = mybir.AluOpType.subtract
    mult = mybir.AluOpType.mult
    add = mybir.AluOpType.add
    is_eq = mybir.AluOpType.is_equal
    axX = mybir.AxisListType.X

    with tile.TileContext(nc) as tc:
        with (
            tc.tile_pool(name="persist", bufs=1) as pp,
            tc.tile_pool(name="xstage", bufs=xbufs) as xp,
            tc.tile_pool(name="work", bufs=wbufs) as wp,
            tc.tile_pool(name="ostage", bufs=obufs) as op,
        ):
            cblob = pp.tile([128, FCW], f32, tag="cblob")
            # consts go on the ACT queue so the SP queue starts the first
            # x-chunk DMA immediately
            nc.scalar.dma_start(out=cblob[:], in_=consts_d[:])
            o = _FOFF
            wrow = cblob[:, o['wrow']:o['wrow'] + V]
            iota20m = cblob[:, o['iota20m']:o['iota20m'] + V]
            rr0 = cblob[:, o['rr0']:o['rr0'] + 1]
            ppc = cblob[:, o['pp']:o['pp'] + 1]
            ppfk = cblob[:, o['ppfk']:o['ppfk'] + 1]
            cadd = cblob[:, o['cadd']:o['cadd'] + 1]
            if barrier:
                tc.strict_bb_all_engine_barrier()

            t0 = 0
            for ncols in sched:
                xt = xp.tile([BLOC, tch, V], f32, tag="xt")
                nc.sync.dma_start(out=xt[:, :ncols, :],
                                  in_=x_d[:, t0:t0 + ncols, :])
                # xm = x * (40 - v): one nonzero (= 40 - a) per (b, t) window
                xm = wp.tile([BLOC, tch, V], xm_dt, tag="xm")
                meng = nc.gpsimd if mult_engine == "gpsimd" else nc.vector
                meng.tensor_tensor(
                    out=xm[:, :ncols, :], in0=xt[:, :ncols, :],
                    in1=wrow[:].unsqueeze(1).broadcast_to((BLOC, ncols, V)),
                    op=mult)
                # a5 = 40 - a  (window max; exact, entries are 0 or 40-a >= 21)
                a5 = wp.tile([BLOC, tch], xm_dt, tag="a5")
                nc.vector.reduce_max(out=a5[:, :ncols], in_=xm[:, :ncols, :],
                                     axis=axX)
                # tp2 = (a5 - (20 - m*)) * p*   in [0, 741]
                tp2 = wp.tile([BLOC, tch], f32, tag="tp2")
                nc.vector.tensor_scalar(
                    out=tp2[:, :ncols], in0=a5[:, :ncols], scalar1=rr0,
                    op0=sub, scalar2=ppc, op1=mult)
                # yf = a5*(p*FK) + (FC - rr0*p*FK) = tp2*FK + FC  (one rounding
                # more than via tp2; error <= 3e-6 vs the 3e-3 floor margin).
                # q1 = RN(yf) = floor(tp2/20) + 1  (exact)
                yf = wp.tile([BLOC, tch], f32, tag="yf")
                nc.vector.tensor_scalar(
                    out=yf[:, :ncols], in0=a5[:, :ncols], scalar1=ppfk,
                    op0=mult, scalar2=cadd, op1=add)
                qf = wp.tile([BLOC, tch], f32, tag="qf")
                nc.vector.tensor_scalar(
                    out=qf[:, :ncols], in0=yf[:, :ncols], scalar1=FM,
                    op0=add, scalar2=FM, op1=sub)
                # um = tp2 - 20*q1 = (tp2 mod 20) - 20   in [-20, -1]
                um = wp.tile([BLOC, tch], f32, tag="um")
                nc.vector.scalar_tensor_tensor(
                    out=um[:, :ncols], in0=qf[:, :ncols], scalar=-20.0,
                    op0=mult, in1=tp2[:, :ncols], op1=add)
                # oh[b,t,v] = (um == v - 20)
                oh = op.tile([BLOC, tch, V], f32, tag="oh")
                nc.vector.tensor_tensor(
                    out=oh[:, :ncols, :],
                    in0=um[:, :ncols].unsqueeze(2).broadcast_to(
                        (BLOC, ncols, V)),
                    in1=iota20m[:].unsqueeze(1).broadcast_to((BLOC, ncols, V)),
                    op=is_eq)
                nc.scalar.dma_start(out=ys_d[:, t0:t0 + ncols, :],
                                    in_=oh[:, :ncols, :])
                t0 += ncols

    nc.finalize()
    return nc


def _build_fast_consts(b, EW, a0):
    """Bootstrap 3 steps on the count formulation; returns per-row frozen
    scalars and the first-3-step symbols for the host patch."""
    f32 = np.float32
    inv = _inv_table()
    batch = a0.shape[0]
    net = np.broadcast_to(b.astype(f32), (batch, J2)).copy()   # net_0 = b
    us = []
    for t in range(TPATCH):
        m = np.argmax(net[:, :V], axis=-1)
        s = np.argmax(net[:, V:], axis=-1)
        u = ((a0[:, t] - m) * inv[s]) % V
        us.append(u)
        net = net + EW[u]
    m = np.argmax(net[:, :V], axis=-1)          # frozen (m*, s*)
    s = np.argmax(net[:, V:], axis=-1)
    pstar = (V - inv[s]) % V
    rr0 = (V - m).astype(f32)                   # 20 - m*
    pp = pstar.astype(f32)
    return rr0, pp, np.stack(us, axis=1)        # [B], [B], [B, TPATCH]


def _make_fast_in_maps(x, rr0, pp, nsteps):
    f32 = np.float32
    in_maps = []
    base = np.zeros((128, FCW), dtype=f32)
    o = _FOFF
    base[:, o['wrow']:o['wrow'] + V] = (40.0 - np.arange(V, dtype=f32))[None, :]
    base[:, o['iota20m']:o['iota20m'] + V] = (np.arange(V, dtype=f32)
                                              - 20.0)[None, :]
    for c in range(NCORES):
        blob = base.copy()
        r = rr0[c * BLOC:(c + 1) * BLOC].astype(np.float64)
        p = pp[c * BLOC:(c + 1) * BLOC].astype(np.float64)
        blob[:, o['rr0']] = rr0[c * BLOC:(c + 1) * BLOC]
        blob[:, o['pp']] = pp[c * BLOC:(c + 1) * BLOC]
        blob[:, o['ppfk']] = (p * FK).astype(f32)
        blob[:, o['cadd']] = (FC - r * (p * FK).astype(f32).astype(np.float64)
                              ).astype(f32)
        xl = np.ascontiguousarray(x[c * BLOC:(c + 1) * BLOC, :nsteps, :], f32)
        in_maps.append(dict(xloc=xl, consts=blob))
    return in_maps


def _verify_fast(out, a, b, EW):
    """Vectorized fixed-point check: the output trajectory is self-consistent
    under the reference recurrence (sufficient: it IS the reference output,
    by induction over t). Non-coprime s is covered by the same formula
    (INV_P maps it to index 0, i.e. one_hot(0), which inv[s]=0 reproduces)."""
    inv = _inv_table()
    batch, nsteps = out.shape[0], out.shape[1]
    if np.count_nonzero(out) != batch * nsteps:
        return False
    if float(np.sum(out)) != float(batch * nsteps):   # all nonzeros exactly 1.0
        return False
    u_dev = np.argmax(out, axis=-1)                       # [B, L]
    rowE = out.reshape(-1, V) @ EW                        # EW[u_t] rows (BLAS)
    rowE = rowE.reshape(batch, nsteps, J2)
    net = np.cumsum(rowE, axis=1) - rowE + b.astype(np.float32)
    m = np.argmax(net[..., :V], axis=-1)
    s = np.argmax(net[..., V:], axis=-1)
    u_chk = ((a - m) * inv[s]) % V
    return bool(np.array_equal(u_chk, u_dev))


# ---------------------------------------------------------------------------
# sequential fallback module (previous baseline, verbatim)
# ---------------------------------------------------------------------------

# column offsets inside the single consts blob [128, CONSTS_W]
_COFF = {'emb32': 0, 'wout': 128, 'brow128': 168, 'cmab': 208, 'c2': 248,
         'iota32': 268, 'c20j': 300, 'ewb32': 320, 'ident': 360}
CONSTS_W = 488


def _build_consts_blob(emb, W_out, b):
    """Host-side constants packed into one [128, CONSTS_W] fp32 blob."""
    f32 = np.float32
    blob = np.zeros((128, CONSTS_W), dtype=f32)
    o = _COFF
    blob[:V, o['emb32']:o['emb32'] + D] = emb
    blob[:, o['wout']:o['wout'] + J2] = W_out
    blob[:, o['brow128']:o['brow128'] + J2] = b[None, :]
    ewb = (emb.astype(np.float64) @ W_out.astype(np.float64)
           + b.astype(np.float64)).astype(f32)
    blob[:V, o['ewb32']:o['ewb32'] + J2] = ewb
    blob[:, o['ident']:o['ident'] + 128] = np.eye(128, dtype=f32)
    inv = _inv_table()
    p = (V - inv) % V
    j = np.arange(V)
    c0 = (V - j).astype(f32)
    blob[:, o['cmab']:o['cmab'] + J2] = np.concatenate([c0, c0])[None, :]
    blob[:, o['c2']:o['c2'] + V] = (c0 + p.astype(f32) / 64.0)[None, :]
    iota32 = np.arange(32, dtype=f32) - 380.0
    iota32[V:] = 1000.0
    blob[:, o['iota32']:o['iota32'] + 32] = iota32[None, :]
    blob[:, o['c20j']:o['c20j'] + V] = (V - np.arange(V, dtype=f32))[None, :]
    return blob


def _build_seq_module(nsteps):
    import concourse.bacc as bacc
    import concourse.mybir as mybir
    import concourse.tile as tile
    from concourse.tile_rust import add_dep_helper

    f32 = mybir.dt.float32
    nc = bacc.Bacc()

    x_d = nc.declare_dram_parameter("xloc", [BLOC, nsteps, V], f32, isOutput=False)
    consts_d = nc.declare_dram_parameter("consts", [128, CONSTS_W], f32,
                                         isOutput=False)
    ys_d = nc.declare_dram_parameter("ys", [BLOC, nsteps, V], f32, isOutput=True)

    XCH = min(128, nsteps)
    nxch = (nsteps + XCH - 1) // XCH
    YCH = min(64, nsteps)
    RING = 2 * YCH

    sub = mybir.AluOpType.subtract
    mult = mybir.AluOpType.mult
    add = mybir.AluOpType.add
    is_ge = mybir.AluOpType.is_ge
    is_eq = mybir.AluOpType.is_equal
    axX = mybir.AxisListType.X

    with tile.TileContext(nc) as tc:
        with (
            tc.tile_pool(name="persist", bufs=1) as pp,
            tc.tile_pool(name="xstage", bufs=2) as xp,
            tc.tile_pool(name="psum", bufs=1, space="PSUM") as pspool,
        ):
            cblob = pp.tile([128, CONSTS_W], f32, tag="cblob")
            nc.sync.dma_start(out=cblob[:], in_=consts_d[:])
            o = _COFF
            emb32 = cblob[0:32, o['emb32']:o['emb32'] + D]
            wout = cblob[:, o['wout']:o['wout'] + J2]
            brow128 = cblob[:, o['brow128']:o['brow128'] + J2]
            cmab = cblob[:, o['cmab']:o['cmab'] + J2]
            c2 = cblob[:, o['c2']:o['c2'] + V]
            iota32 = cblob[:, o['iota32']:o['iota32'] + 32]
            c20j = cblob[:, o['c20j']:o['c20j'] + V]
            ewb32 = cblob[0:32, o['ewb32']:o['ewb32'] + J2]
            ident = cblob[:, o['ident']:o['ident'] + 128]
            tc.strict_bb_all_engine_barrier()

            a5 = pp.tile([BLOC, nsteps], f32, tag="a5")
            ysring = pp.tile([BLOC, RING, 32], f32, tag="ysring")
            stateT = pp.tile([D, BLOC], f32, tag="stateT")
            dfm = pp.tile([BLOC, J2], f32, tag="dfm")
            msk = pp.tile([BLOC, 3 * V], f32, tag="msk")
            mx = pp.tile([BLOC, 2], f32, tag="mx")
            rr3 = pp.tile([BLOC, 3], f32, tag="rr3")
            pf = pp.tile([BLOC, 1], f32, tag="pf")
            tpf = pp.tile([BLOC, 1], f32, tag="tpf")
            yf = pp.tile([BLOC, 1], f32, tag="yf")
            qf = pp.tile([BLOC, 1], f32, tag="qf")
            q20f = pp.tile([BLOC, 1], f32, tag="q20f")
            ohT = pp.tile([32, BLOC], f32, tag="ohT")

            net_ps = [pspool.tile([BLOC, 512], f32, tag=f"net_ps{i}",
                                  name=f"net_ps{i}")[:, 0:J2]
                      for i in range(2)]
            ohT_ps = pspool.tile([32, BLOC], f32, tag="ohT_ps")
            state_ps = pspool.tile([D, BLOC], f32, tag="state_ps")

            nc.gpsimd.memset(stateT[:], 0.0)
            nc.vector.tensor_copy(out=net_ps[0][:], in_=brow128[:])

            for c in range(nxch):
                t0 = c * XCH
                ncols = min(XCH, nsteps - t0)
                xt = xp.tile([BLOC, XCH, V], f32, tag="xt")
                xm = xp.tile([BLOC, XCH, V], f32, tag="xm")
                nc.sync.dma_start(out=xt[:, :ncols, :], in_=x_d[:, t0:t0 + ncols, :])
                nc.vector.tensor_tensor(
                    out=xm[:, :ncols, :], in0=xt[:, :ncols, :],
                    in1=c20j[:].unsqueeze(1).broadcast_to((BLOC, ncols, V)),
                    op=mult)
                nc.vector.reduce_sum(out=a5[:, t0:t0 + ncols], in_=xm[:, :ncols, :],
                                     axis=axX)

            for t in range(nsteps):
                nA = net_ps[t % 2]
                nB = net_ps[(t + 1) % 2]

                nc.vector.reduce_max(out=mx[:], in_=nA[:].rearrange(
                    "b (h v) -> b h v", h=2), axis=axX)
                nc.vector.tensor_tensor(
                    out=dfm[:].rearrange("b (h v) -> b h v", h=2),
                    in0=nA[:].rearrange("b (h v) -> b h v", h=2),
                    in1=mx[:].unsqueeze(2).broadcast_to((BLOC, 2, V)),
                    op=sub)
                nc.vector.scalar_tensor_tensor(
                    out=msk[:, 0:J2], in0=dfm[:], scalar=0.0, in1=cmab[:],
                    op0=is_ge, op1=mult)
                nc.vector.scalar_tensor_tensor(
                    out=msk[:, J2:3 * V], in0=dfm[:, V:J2], scalar=0.0,
                    in1=c2[:], op0=is_ge, op1=mult)
                nc.vector.reduce_max(out=rr3[:], in_=msk[:].rearrange(
                    "b (h v) -> b h v", h=3), axis=axX)
                nc.vector.tensor_scalar(
                    out=pf[:], in0=rr3[:, 2:3], scalar1=rr3[:, 1:2], op0=sub,
                    scalar2=64.0, op1=mult)
                nc.vector.tensor_scalar(
                    out=tpf[:], in0=a5[:, t:t + 1], scalar1=rr3[:, 0:1],
                    op0=sub, scalar2=pf[:], op1=mult)
                nc.vector.tensor_scalar(
                    out=yf[:], in0=tpf[:], scalar1=0.0500030517578125,
                    op0=mult, scalar2=18.50025177001953125, op1=add)
                nc.vector.tensor_scalar(
                    out=qf[:], in0=yf[:], scalar1=8388608.0, op0=add,
                    scalar2=8388608.0, op1=sub)
                nc.vector.tensor_scalar(
                    out=q20f[:], in0=qf[:], scalar1=20.0, scalar2=None,
                    op0=mult)
                ohs = ysring[:, t % RING, :]
                nc.vector.tensor_scalar(
                    out=ohs, in0=iota32[:], scalar1=q20f[:], op0=add,
                    scalar2=tpf[:], op1=is_eq)
                if t > 0:
                    nc.scalar.copy(out=stateT[:], in_=state_ps[:])

                mm1 = nc.tensor.matmul(nB[:], stateT[:], wout[:],
                                       start=True, stop=False,
                                       skip_group_check=True)

                nc.tensor.transpose(ohT_ps[:], ohs, ident[:])
                nc.vector.tensor_copy(out=ohT[:], in_=ohT_ps[:])

                m4 = nc.tensor.matmul(nB[:], ohT[:], ewb32[:],
                                      start=False, stop=True,
                                      skip_group_check=True)
                add_dep_helper(m4.ins, mm1.ins, sync=False,
                               reason="net accum order")
                nc.tensor.matmul(state_ps[:], emb32[:], ohT[:],
                                 start=(t == 0), stop=(t == nsteps - 1),
                                 skip_group_check=True)

                if (t + 1) % YCH == 0:
                    h0 = (t + 1 - YCH) % RING
                    nc.sync.dma_start(
                        out=ys_d[:, t + 1 - YCH:t + 1, :],
                        in_=ysring[:, h0:h0 + YCH, :V])
            if nsteps % YCH:
                tdone = (nsteps // YCH) * YCH
                h0 = tdone % RING
                nc.sync.dma_start(
                    out=ys_d[:, tdone:nsteps, :],
                    in_=ysring[:, h0:h0 + (nsteps - tdone), :V])

    nc.finalize()
    return nc


def _make_seq_in_maps(x, emb, W_out, b, nsteps):
    f32 = np.float32
    blob = _build_consts_blob(np.asarray(emb, f32), np.asarray(W_out, f32),
                              np.asarray(b, f32))
    in_maps = []
    for c in range(NCORES):
        xl = np.ascontiguousarray(x[c * BLOC:(c + 1) * BLOC, :nsteps, :], f32)
        in_maps.append(dict(xloc=xl, consts=blob))
    return in_maps


# ---------------------------------------------------------------------------
# host driver
# ---------------------------------------------------------------------------

def _get_module(nsteps):
    key = ("fast", nsteps)
    if key not in _CACHE:
        _CACHE[key] = _build_fast_module(nsteps)
    return _CACHE[key]


def _get_seq_module(nsteps):
    key = ("seq", nsteps)
    if key not in _CACHE:
        _CACHE[key] = _build_seq_module(nsteps)
    return _CACHE[key]


def _run_spmd(nc, in_maps):
    from concourse.bass_utils import run_bass_kernel_spmd
    # the axon-tunneled device occasionally reports a transient
    # "unrecoverable" execution failure; one retry has been observed to
    # succeed
    last_err = None
    for _ in range(3):
        try:
            return run_bass_kernel_spmd(nc, in_maps, list(range(NCORES)))
        except Exception as e:  # jax.errors.JaxRuntimeError and friends
            last_err = e
    raise last_err


def _make_in_maps(x, emb, W_out, b, nsteps):
    """Fast-path in_maps (kept for test.py compatibility)."""
    f32 = np.float32
    EW = (np.asarray(emb, np.float64) @ np.asarray(W_out, np.float64)).astype(f32)
    a = np.argmax(np.asarray(x, f32), axis=-1)
    rr0, pp, _ = _build_fast_consts(np.asarray(b, f32), EW, a)
    return _make_fast_in_maps(np.asarray(x, f32), rr0, pp, nsteps)


def kernel(x, emb, W_out, b):
    f32 = np.float32
    x = np.asarray(x, f32)
    emb = np.asarray(emb, f32)
    W_out = np.asarray(W_out, f32)
    b = np.asarray(b, f32)

    nsteps = x.shape[1]
    EW = (emb.astype(np.float64) @ W_out.astype(np.float64)).astype(f32)
    a = np.argmax(x, axis=-1)                       # [B, L] symbol indices

    rr0, pp, us_early = _build_fast_consts(b, EW, a)
    nc = _get_module(nsteps)
    in_maps = _make_fast_in_maps(x, rr0, pp, nsteps)
    res = _run_spmd(nc, in_maps)
    out = np.concatenate([res.results[c]["ys"] for c in range(NCORES)], axis=0)
    # patch the pre-freeze steps computed on host
    npatch = min(TPATCH, nsteps)
    eye = np.eye(V, dtype=f32)
    out[:, :npatch, :] = eye[us_early[:, :npatch]]

    if _verify_fast(out, a, b, EW):
        return out.astype(f32)

    # attractor assumption failed for these inputs: run the general
    # sequential module instead
    nc = _get_seq_module(nsteps)
    in_maps = _make_seq_in_maps(x, emb, W_out, b, nsteps)
    res = _run_spmd(nc, in_maps)
    out = np.concatenate([res.results[c]["ys"] for c in range(NCORES)], axis=0)
    return out.astype(f32)


# revision 30
# speedup vs baseline: 1.0221x; 1.0221x over previous
"""Trainium2 Bass kernel for the DiscreteAutoregressiveFlow sampling problem.

Reference semantics (B=1024, L=1024, V=20, D=128):
    scan over t:  net = state @ W_out + b          [B, 2V]
                  m = argmax(net[:, :V]); s = argmax(net[:, V:])
                  u = ((a_t - m) * inv(s)) % V     (a_t = index of one-hot x_t,
                                                    inv(s) = mult. inverse mod V,
                                                    0 if s not coprime with V)
                  out_t = one_hot(u); state += emb[u]
Outputs ys[b, t] = one_hot(u_t).

Key structural property (exploited, then verified post-hoc): net_t depends on
the history only through the COUNT vector c_t of previously emitted symbols
(net_t = b + c_t @ (emb @ W_out)), and the dynamics have a self-reinforcing
attractor: the argmax pair (m_t, s_t) stops changing after t=2 and its margin
grows ~linearly in t (min margin 3.6 at t=32, 119 at t=1023 on the reference
input distribution). Hence for t >= 2 the scan collapses to the elementwise
map  u_t = ((a_t - m*) * p*) mod V  with per-row frozen (m*, p*).

Device kernel (pure data-parallel streaming, per core 128 batch rows):
    a5   = 40 - a_t              (weighted max over the one-hot x chunk)
    tp2  = (a5 - (20 - m*)) * p* = (20 + m* - a_t) * p*   in [0, 741]
    q    = floor(tp2 / 20)       (fp32 2^23 round-to-nearest trick, exact —
                                  constants validated exhaustively)
    oh   = one_hot(tp2 - 20q)    (is_eq against iota, written to ys)
The host computes the per-row frozen (m*, p*) by a 3-step bootstrap (tiny,
O(B*V) numpy - same spirit as the host-built emb @ W_out table), patches the
first 3 output steps, and then VERIFIES the full trajectory in vectorized
numpy: given the device output u, the recurrence check
    argmax(b + exclusive-cumsum(onehot(u)) @ EW)  ->  u
is embarrassingly parallel, and any self-consistent trajectory is (by
induction over t) THE unique reference trajectory. On verification failure
the slow-but-general sequential module (previous baseline, kept verbatim
below) is built and used instead.
"""

import numpy as np

B, L, V, D = 1024, 1024, 20, 128
NCORES = 8
BLOC = B // NCORES  # 128 batch rows per core
J2 = 2 * V          # 40

_CACHE = {}

# ---------------------------------------------------------------------------
# fast streaming module
# ---------------------------------------------------------------------------

# consts blob [128, FCW] column layout
_FOFF = {'wrow': 0, 'iota20m': 20, 'rr0': 40, 'pp': 41, 'ppfk': 42, 'cadd': 43}
FCW = 44

# floor((tp2)/20) via round-to-nearest: q1 = RN(tp2*FK + FC) = floor(tp2/20)+1
# for integer tp2 in [0, 741]; validated exhaustively (FC in [0.505, 0.547]).
FK = 3277.0 / 65536.0
FC = 0.525
FM = 8388608.0  # 2^23

TCH = 80                    # steps per chunk (DMA/compute overlap sweet spot)
TPATCH = 3                  # host-patched leading steps
XM_BF16 = True              # bf16 intermediate for the weighted-max pass
MULT_ENGINE = "gpsimd"      # engine for the x*w pass: "gpsimd" | "vector"


def _inv_table():
    inv = np.zeros(V, dtype=np.int64)
    for k in range(1, V):
        if np.gcd(k, V) == 1:
            inv[k] = pow(k, -1, V)
    return inv


def _chunk_schedule(nsteps, tch, ramp, tail=0):
    """Chunk sizes: optionally smaller leading/trailing chunks to shorten the
    pipeline fill/drain, full-width chunks in the middle. `tail` may be an
    int or a tuple of trailing chunk sizes (largest first)."""
    sizes = []
    left = nsteps
    if ramp and nsteps > tch:
        for r in ((ramp,) if isinstance(ramp, int) else tuple(ramp)):
            r = min(r, left)
            if r > 0:
                sizes.append(r)
                left -= r
    tails = []
    if nsteps > tch:
        for t in ((tail,) if isinstance(tail, int) else tuple(tail)):
            t = min(t, left)
            if t > 0:
                tails.append(t)
                left -= t
    while left > 0:
        s = min(tch, left)
        sizes.append(s)
        left -= s
    sizes.extend(tails)
    return sizes


def _build_fast_module(nsteps, tch=None, xm_bf16=None, mult_engine=None,
                       xbufs=6, wbufs=4, obufs=6, ramp=40, tail=(40, 16, 8),
                       barrier=False, dve_tail_n=0):
    import concourse.bacc as bacc
    import concourse.mybir as mybir
    import concourse.tile as tile

    if tch is None:
        tch = TCH
    if xm_bf16 is None:
        xm_bf16 = XM_BF16
    if mult_engine is None:
        mult_engine = MULT_ENGINE
    f32 = mybir.dt.float32
    xm_dt = mybir.dt.bfloat16 if xm_bf16 else f32
    nc = bacc.Bacc()

    x_d = nc.declare_dram_parameter("xloc", [BLOC, nsteps, V], f32, isOutput=False)
    consts_d = nc.declare_dram_parameter("consts", [128, FCW], f32, isOutput=False)
    ys_d = nc.declare_dram_parameter("ys", [BLOC, nsteps, V], f32, isOutput=True)

    tch = min(tch, nsteps)
    sched = _chunk_schedule(nsteps, tch, ramp, tail)

    sub = mybir.AluOpType.subtract
    mult = mybir.AluOpType.mult
    add = mybir.AluOpType.add
    is_eq = mybir.AluOpType.is_equal
    axX = mybir.AxisListType.X

    with tile.TileContext(nc) as tc:
        with (
            tc.tile_pool(name="persist", bufs=1) as pp,
            tc.tile_pool(name="xstage", bufs=xbufs) as xp,
            tc.tile_pool(name="work", bufs=wbufs) as wp,
            tc.tile_pool(name="ostage", bufs=obufs) as op,
        ):
            cblob = pp.tile([128, FCW], f32, tag="cblob")
            # consts go on the ACT queue so the SP queue starts the first
            # x-chunk DMA immediately
            nc.scalar.dma_start(out=cblob[:], in_=consts_d[:])
            o = _FOFF
            wrow = cblob[:, o['wrow']:o['wrow'] + V]
            iota20m = cblob[:, o['iota20m']:o['iota20m'] + V]
            rr0 = cblob[:, o['rr0']:o['rr0'] + 1]
            ppc = cblob[:, o['pp']:o['pp'] + 1]
            ppfk = cblob[:, o['ppfk']:o['ppfk'] + 1]
            cadd = cblob[:, o['cadd']:o['cadd'] + 1]
            if barrier:
                tc.strict_bb_all_engine_barrier()

            t0 = 0
            for ci, ncols in enumerate(sched):
                xt = xp.tile([BLOC, tch, V], f32, tag="xt")
                nc.sync.dma_start(out=xt[:, :ncols, :],
                                  in_=x_d[:, t0:t0 + ncols, :])
                # xm = x * (40 - v): one nonzero (= 40 - a) per (b, t) window
                # (trailing drain chunks may run the mult on DVE to drop the
                # gpsimd handoff from the critical drain path)
                on_dve = (mult_engine != "gpsimd"
                          or ci >= len(sched) - dve_tail_n)
                xm = wp.tile([BLOC, tch, V], xm_dt, tag="xm")
                meng = nc.vector if on_dve else nc.gpsimd
                meng.tensor_tensor(
                    out=xm[:, :ncols, :], in0=xt[:, :ncols, :],
                    in1=wrow[:].unsqueeze(1).broadcast_to((BLOC, ncols, V)),
                    op=mult)
                # a5 = 40 - a  (window max; exact, entries are 0 or 40-a >= 21)
                a5 = wp.tile([BLOC, tch], xm_dt, tag="a5")
                nc.vector.reduce_max(out=a5[:, :ncols], in_=xm[:, :ncols, :],
                                     axis=axX)
                # tp2 = (a5 - (20 - m*)) * p*   in [0, 741]
                tp2 = wp.tile([BLOC, tch], f32, tag="tp2")
                nc.vector.tensor_scalar(
                    out=tp2[:, :ncols], in0=a5[:, :ncols], scalar1=rr0,
                    op0=sub, scalar2=ppc, op1=mult)
                # yf = a5*(p*FK) + (FC - rr0*p*FK) = tp2*FK + FC  (one rounding
                # more than via tp2; error <= 3e-6 vs the 3e-3 floor margin).
                # q1 = RN(yf) = floor(tp2/20) + 1  (exact)
                yf = wp.tile([BLOC, tch], f32, tag="yf")
                nc.vector.tensor_scalar(
                    out=yf[:, :ncols], in0=a5[:, :ncols], scalar1=ppfk,
                    op0=mult, scalar2=cadd, op1=add)
                qf = wp.tile([BLOC, tch], f32, tag="qf")
                nc.vector.tensor_scalar(
                    out=qf[:, :ncols], in0=yf[:, :ncols], scalar1=FM,
                    op0=add, scalar2=FM, op1=sub)
                # um = tp2 - 20*q1 = (tp2 mod 20) - 20   in [-20, -1]
                um = wp.tile([BLOC, tch], f32, tag="um")
                nc.vector.scalar_tensor_tensor(
                    out=um[:, :ncols], in0=qf[:, :ncols], scalar=-20.0,
                    op0=mult, in1=tp2[:, :ncols], op1=add)
                # oh[b,t,v] = (um == v - 20)
                oh = op.tile([BLOC, tch, V], f32, tag="oh")
                nc.vector.tensor_tensor(
                    out=oh[:, :ncols, :],
                    in0=um[:, :ncols].unsqueeze(2).broadcast_to(
                        (BLOC, ncols, V)),
                    in1=iota20m[:].unsqueeze(1).broadcast_to((BLOC, ncols, V)),
                    op=is_eq)
                nc.scalar.dma_start(out=ys_d[:, t0:t0 + ncols, :],
                                    in_=oh[:, :ncols, :])
                t0 += ncols

    nc.finalize()
    return nc


def _build_fast_consts(b, EW, a0):
    """Bootstrap 3 steps on the count formulation; returns per-row frozen
    scalars and the first-3-step symbols for the host patch."""
    f32 = np.float32
    inv = _inv_table()
    batch = a0.shape[0]
    net = np.broadcast_to(b.astype(f32), (batch, J2)).copy()   # net_0 = b
    us = []
    for t in range(TPATCH):
        m = np.argmax(net[:, :V], axis=-1)
        s = np.argmax(net[:, V:], axis=-1)
        u = ((a0[:, t] - m) * inv[s]) % V
        us.append(u)
        net = net + EW[u]
    m = np.argmax(net[:, :V], axis=-1)          # frozen (m*, s*)
    s = np.argmax(net[:, V:], axis=-1)
    pstar = (V - inv[s]) % V
    rr0 = (V - m).astype(f32)                   # 20 - m*
    pp = pstar.astype(f32)
    return rr0, pp, np.stack(us, axis=1)        # [B], [B], [B, TPATCH]


def _make_fast_in_maps(x, rr0, pp, nsteps):
    f32 = np.float32
    in_maps = []
    base = np.zeros((128, FCW), dtype=f32)
    o = _FOFF
    base[:, o['wrow']:o['wrow'] + V] = (40.0 - np.arange(V, dtype=f32))[None, :]
    base[:, o['iota20m']:o['iota20m'] + V] = (np.arange(V, dtype=f32)
                                              - 20.0)[None, :]
    for c in range(NCORES):
        blob = base.copy()
        r = rr0[c * BLOC:(c + 1) * BLOC].astype(np.float64)
        p = pp[c * BLOC:(c + 1) * BLOC].astype(np.float64)
        blob[:, o['rr0']] = rr0[c * BLOC:(c + 1) * BLOC]
        blob[:, o['pp']] = pp[c * BLOC:(c + 1) * BLOC]
        blob[:, o['ppfk']] = (p * FK).astype(f32)
        blob[:, o['cadd']] = (FC - r * (p * FK).astype(f32).astype(np.float64)
                              ).astype(f32)
        xl = np.ascontiguousarray(x[c * BLOC:(c + 1) * BLOC, :nsteps, :], f32)
        in_maps.append(dict(xloc=xl, consts=blob))
    return in_maps


def _verify_fast(out, a, b, EW):
    """Vectorized fixed-point check: the output trajectory is self-consistent
    under the reference recurrence (sufficient: it IS the reference output,
    by induction over t). Non-coprime s is covered by the same formula
    (INV_P maps it to index 0, i.e. one_hot(0), which inv[s]=0 reproduces)."""
    inv = _inv_table()
    batch, nsteps = out.shape[0], out.shape[1]
    if np.count_nonzero(out) != batch * nsteps:
        return False
    if float(np.sum(out)) != float(batch * nsteps):   # all nonzeros exactly 1.0
        return False
    u_dev = np.argmax(out, axis=-1)                       # [B, L]
    rowE = out.reshape(-1, V) @ EW                        # EW[u_t] rows (BLAS)
    rowE = rowE.reshape(batch, nsteps, J2)
    net = np.cumsum(rowE, axis=1) - rowE + b.astype(np.float32)
    m = np.argmax(net[..., :V], axis=-1)
    s = np.argmax(net[..., V:], axis=-1)
    u_chk = ((a - m) * inv[s]) % V
    return bool(np.array_equal(u_chk, u_dev))


# ---------------------------------------------------------------------------
# sequential fallback module (previous baseline, verbatim)
# ---------------------------------------------------------------------------

# column offsets inside the single consts blob [128, CONSTS_W]
_COFF = {'emb32': 0, 'wout': 128, 'brow128': 168, 'cmab': 208, 'c2': 248,
         'iota32': 268, 'c20j': 300, 'ewb32': 320, 'ident': 360}
CONSTS_W = 488


def _build_consts_blob(emb, W_out, b):
    """Host-side constants packed into one [128, CONSTS_W] fp32 blob."""
    f32 = np.float32
    blob = np.zeros((128, CONSTS_W), dtype=f32)
    o = _COFF
    blob[:V, o['emb32']:o['emb32'] + D] = emb
    blob[:, o['wout']:o['wout'] + J2] = W_out
    blob[:, o['brow128']:o['brow128'] + J2] = b[None, :]
    ewb = (emb.astype(np.float64) @ W_out.astype(np.float64)
           + b.astype(np.float64)).astype(f32)
    blob[:V, o['ewb32']:o['ewb32'] + J2] = ewb
    blob[:, o['ident']:o['ident'] + 128] = np.eye(128, dtype=f32)
    inv = _inv_table()
    p = (V - inv) % V
    j = np.arange(V)
    c0 = (V - j).astype(f32)
    blob[:, o['cmab']:o['cmab'] + J2] = np.concatenate([c0, c0])[None, :]
    blob[:, o['c2']:o['c2'] + V] = (c0 + p.astype(f32) / 64.0)[None, :]
    iota32 = np.arange(32, dtype=f32) - 380.0
    iota32[V:] = 1000.0
    blob[:, o['iota32']:o['iota32'] + 32] = iota32[None, :]
    blob[:, o['c20j']:o['c20j'] + V] = (V - np.arange(V, dtype=f32))[None, :]
    return blob


def _build_seq_module(nsteps):
    import concourse.bacc as bacc
    import concourse.mybir as mybir
    import concourse.tile as tile
    from concourse.tile_rust import add_dep_helper

    f32 = mybir.dt.float32
    nc = bacc.Bacc()

    x_d = nc.declare_dram_parameter("xloc", [BLOC, nsteps, V], f32, isOutput=False)
    consts_d = nc.declare_dram_parameter("consts", [128, CONSTS_W], f32,
                                         isOutput=False)
    ys_d = nc.declare_dram_parameter("ys", [BLOC, nsteps, V], f32, isOutput=True)

    XCH = min(128, nsteps)
    nxch = (nsteps + XCH - 1) // XCH
    YCH = min(64, nsteps)
    RING = 2 * YCH

    sub = mybir.AluOpType.subtract
    mult = mybir.AluOpType.mult
    add = mybir.AluOpType.add
    is_ge = mybir.AluOpType.is_ge
    is_eq = mybir.AluOpType.is_equal
    axX = mybir.AxisListType.X

    with tile.TileContext(nc) as tc:
        with (
            tc.tile_pool(name="persist", bufs=1) as pp,
            tc.tile_pool(name="xstage", bufs=2) as xp,
            tc.tile_pool(name="psum", bufs=1, space="PSUM") as pspool,
        ):
            cblob = pp.tile([128, CONSTS_W], f32, tag="cblob")
            nc.sync.dma_start(out=cblob[:], in_=consts_d[:])
            o = _COFF
            emb32 = cblob[0:32, o['emb32']:o['emb32'] + D]
            wout = cblob[:, o['wout']:o['wout'] + J2]
            brow128 = cblob[:, o['brow128']:o['brow128'] + J2]
            cmab = cblob[:, o['cmab']:o['cmab'] + J2]
            c2 = cblob[:, o['c2']:o['c2'] + V]
            iota32 = cblob[:, o['iota32']:o['iota32'] + 32]
            c20j = cblob[:, o['c20j']:o['c20j'] + V]
            ewb32 = cblob[0:32, o['ewb32']:o['ewb32'] + J2]
            ident = cblob[:, o['ident']:o['ident'] + 128]
            tc.strict_bb_all_engine_barrier()

            a5 = pp.tile([BLOC, nsteps], f32, tag="a5")
            ysring = pp.tile([BLOC, RING, 32], f32, tag="ysring")
            stateT = pp.tile([D, BLOC], f32, tag="stateT")
            dfm = pp.tile([BLOC, J2], f32, tag="dfm")
            msk = pp.tile([BLOC, 3 * V], f32, tag="msk")
            mx = pp.tile([BLOC, 2], f32, tag="mx")
            rr3 = pp.tile([BLOC, 3], f32, tag="rr3")
            pf = pp.tile([BLOC, 1], f32, tag="pf")
            tpf = pp.tile([BLOC, 1], f32, tag="tpf")
            yf = pp.tile([BLOC, 1], f32, tag="yf")
            qf = pp.tile([BLOC, 1], f32, tag="qf")
            q20f = pp.tile([BLOC, 1], f32, tag="q20f")
            ohT = pp.tile([32, BLOC], f32, tag="ohT")

            net_ps = [pspool.tile([BLOC, 512], f32, tag=f"net_ps{i}",
                                  name=f"net_ps{i}")[:, 0:J2]
                      for i in range(2)]
            ohT_ps = pspool.tile([32, BLOC], f32, tag="ohT_ps")
            state_ps = pspool.tile([D, BLOC], f32, tag="state_ps")

            nc.gpsimd.memset(stateT[:], 0.0)
            nc.vector.tensor_copy(out=net_ps[0][:], in_=brow128[:])

            for c in range(nxch):
                t0 = c * XCH
                ncols = min(XCH, nsteps - t0)
                xt = xp.tile([BLOC, XCH, V], f32, tag="xt")
                xm = xp.tile([BLOC, XCH, V], f32, tag="xm")
                nc.sync.dma_start(out=xt[:, :ncols, :], in_=x_d[:, t0:t0 + ncols, :])
                nc.vector.tensor_tensor(
                    out=xm[:, :ncols, :], in0=xt[:, :ncols, :],
                    in1=c20j[:].unsqueeze(1).broadcast_to((BLOC, ncols, V)),
                    op=mult)
                nc.vector.reduce_sum(out=a5[:, t0:t0 + ncols], in_=xm[:, :ncols, :],
                                     axis=axX)

            for t in range(nsteps):
                nA = net_ps[t % 2]
                nB = net_ps[(t + 1) % 2]

                nc.vector.reduce_max(out=mx[:], in_=nA[:].rearrange(
                    "b (h v) -> b h v", h=2), axis=axX)
                nc.vector.tensor_tensor(
                    out=dfm[:].rearrange("b (h v) -> b h v", h=2),
                    in0=nA[:].rearrange("b (h v) -> b h v", h=2),
                    in1=mx[:].unsqueeze(2).broadcast_to((BLOC, 2, V)),
                    op=sub)
                nc.vector.scalar_tensor_tensor(
                    out=msk[:, 0:J2], in0=dfm[:], scalar=0.0, in1=cmab[:],
                    op0=is_ge, op1=mult)
                nc.vector.scalar_tensor_tensor(
                    out=msk[:, J2:3 * V], in0=dfm[:, V:J2], scalar=0.0,
                    in1=c2[:], op0=is_ge, op1=mult)
                nc.vector.reduce_max(out=rr3[:], in_=msk[:].rearrange(
                    "b (h v) -> b h v", h=3), axis=axX)
                nc.vector.tensor_scalar(
                    out=pf[:], in0=rr3[:, 2:3], scalar1=rr3[:, 1:2], op0=sub,
                    scalar2=64.0, op1=mult)
                nc.vector.tensor_scalar(
                    out=tpf[:], in0=a5[:, t:t + 1], scalar1=rr3[:, 0:1],
                    op0=sub, scalar2=pf[:], op1=mult)
                nc.vector.tensor_scalar(
                    out=yf[:], in0=tpf[:], scalar1=0.0500030517578125,
                    op0=mult, scalar2=18.50025177001953125, op1=add)
                nc.vector.tensor_scalar(
                    out=qf[:], in0=yf[:], scalar1=8388608.0, op0=add,
                    scalar2=8388608.0, op1=sub)
                nc.vector.tensor_scalar(
                    out=q20f[:], in0=qf[:], scalar1=20.0, scalar2=None,
                    op0=mult)
                ohs = ysring[:, t % RING, :]
                nc.vector.tensor_scalar(
                    out=ohs, in0=iota32[:], scalar1=q20f[:], op0=add,
                    scalar2=tpf[:], op1=is_eq)
                if t > 0:
                    nc.scalar.copy(out=stateT[:], in_=state_ps[:])

                mm1 = nc.tensor.matmul(nB[:], stateT[:], wout[:],
                                       start=True, stop=False,
                                       skip_group_check=True)

                nc.tensor.transpose(ohT_ps[:], ohs, ident[:])
                nc.vector.tensor_copy(out=ohT[:], in_=ohT_ps[:])

                m4 = nc.tensor.matmul(nB[:], ohT[:], ewb32[:],
                                      start=False, stop=True,
                                      skip_group_check=True)
                add_dep_helper(m4.ins, mm1.ins, sync=False,
                               reason="net accum order")
                nc.tensor.matmul(state_ps[:], emb32[:], ohT[:],
                                 start=(t == 0), stop=(t == nsteps - 1),
                                 skip_group_check=True)

                if (t + 1) % YCH == 0:
                    h0 = (t + 1 - YCH) % RING
                    nc.sync.dma_start(
                        out=ys_d[:, t + 1 - YCH:t + 1, :],
                        in_=ysring[:, h0:h0 + YCH, :V])
            if nsteps % YCH:
                tdone = (nsteps // YCH) * YCH
                h0 = tdone % RING
                nc.sync.dma_start(
                    out=ys_d[:, tdone:nsteps, :],
                    in_=ysring[:, h0:h0 + (nsteps - tdone), :V])

    nc.finalize()
    return nc


def _make_seq_in_maps(x, emb, W_out, b, nsteps):
    f32 = np.float32
    blob = _build_consts_blob(np.asarray(emb, f32), np.asarray(W_out, f32),
                              np.asarray(b, f32))
    in_maps = []
    for c in range(NCORES):
        xl = np.ascontiguousarray(x[c * BLOC:(c + 1) * BLOC, :nsteps, :], f32)
        in_maps.append(dict(xloc=xl, consts=blob))
    return in_maps


# ---------------------------------------------------------------------------
# host driver
# ---------------------------------------------------------------------------

def _get_module(nsteps):
    key = ("fast", nsteps)
    if key not in _CACHE:
        _CACHE[key] = _build_fast_module(nsteps)
    return _CACHE[key]


def _get_seq_module(nsteps):
    key = ("seq", nsteps)
    if key not in _CACHE:
        _CACHE[key] = _build_seq_module(nsteps)
    return _CACHE[key]


def _run_spmd(nc, in_maps):
    from concourse.bass_utils import run_bass_kernel_spmd
    # the axon-tunneled device occasionally reports a transient
    # "unrecoverable" execution failure; one retry has been observed to
    # succeed
    last_err = None
    for _ in range(3):
        try:
            return run_bass_kernel_spmd(nc, in_maps, list(range(NCORES)))
        except Exception as e:  # jax.errors.JaxRuntimeError and friends
            last_err = e
    raise last_err


def _make_in_maps(x, emb, W_out, b, nsteps):
    """Fast-path in_maps (kept for test.py compatibility)."""
    f32 = np.float32
    EW = (np.asarray(emb, np.float64) @ np.asarray(W_out, np.float64)).astype(f32)
    a = np.argmax(np.asarray(x, f32), axis=-1)
    rr0, pp, _ = _build_fast_consts(np.asarray(b, f32), EW, a)
    return _make_fast_in_maps(np.asarray(x, f32), rr0, pp, nsteps)


def kernel(x, emb, W_out, b):
    f32 = np.float32
    x = np.asarray(x, f32)
    emb = np.asarray(emb, f32)
    W_out = np.asarray(W_out, f32)
    b = np.asarray(b, f32)

    nsteps = x.shape[1]
    EW = (emb.astype(np.float64) @ W_out.astype(np.float64)).astype(f32)
    a = np.argmax(x, axis=-1)                       # [B, L] symbol indices

    rr0, pp, us_early = _build_fast_consts(b, EW, a)
    nc = _get_module(nsteps)
    in_maps = _make_fast_in_maps(x, rr0, pp, nsteps)
    res = _run_spmd(nc, in_maps)
    out = np.concatenate([res.results[c]["ys"] for c in range(NCORES)], axis=0)
    # patch the pre-freeze steps computed on host
    npatch = min(TPATCH, nsteps)
    eye = np.eye(V, dtype=f32)
    out[:, :npatch, :] = eye[us_early[:, :npatch]]

    if _verify_fast(out, a, b, EW):
        return out.astype(f32)

    # attractor assumption failed for these inputs: run the general
    # sequential module instead
    nc = _get_seq_module(nsteps)
    in_maps = _make_seq_in_maps(x, emb, W_out, b, nsteps)
    res = _run_spmd(nc, in_maps)
    out = np.concatenate([res.results[c]["ys"] for c in range(NCORES)], axis=0)
    return out.astype(f32)


# revision 32
# speedup vs baseline: 1.0236x; 1.0015x over previous
"""Trainium2 Bass kernel for the DiscreteAutoregressiveFlow sampling problem.

Reference semantics (B=1024, L=1024, V=20, D=128):
    scan over t:  net = state @ W_out + b          [B, 2V]
                  m = argmax(net[:, :V]); s = argmax(net[:, V:])
                  u = ((a_t - m) * inv(s)) % V     (a_t = index of one-hot x_t,
                                                    inv(s) = mult. inverse mod V,
                                                    0 if s not coprime with V)
                  out_t = one_hot(u); state += emb[u]
Outputs ys[b, t] = one_hot(u_t).

Key structural property (exploited, then verified post-hoc): net_t depends on
the history only through the COUNT vector c_t of previously emitted symbols
(net_t = b + c_t @ (emb @ W_out)), and the dynamics have a self-reinforcing
attractor: the argmax pair (m_t, s_t) stops changing after t=2 and its margin
grows ~linearly in t (min margin 3.6 at t=32, 119 at t=1023 on the reference
input distribution). Hence for t >= 2 the scan collapses to the elementwise
map  u_t = ((a_t - m*) * p*) mod V  with per-row frozen (m*, p*).

Device kernel (pure data-parallel streaming, per core 128 batch rows):
    a5   = 40 - a_t              (weighted max over the one-hot x chunk)
    tp2  = (a5 - (20 - m*)) * p* = (20 + m* - a_t) * p*   in [0, 741]
    q    = floor(tp2 / 20)       (fp32 2^23 round-to-nearest trick, exact —
                                  constants validated exhaustively)
    oh   = one_hot(tp2 - 20q)    (is_eq against iota, written to ys)
The host computes the per-row frozen (m*, p*) by a 3-step bootstrap (tiny,
O(B*V) numpy - same spirit as the host-built emb @ W_out table), patches the
first 3 output steps, and then VERIFIES the full trajectory in vectorized
numpy: given the device output u, the recurrence check
    argmax(b + exclusive-cumsum(onehot(u)) @ EW)  ->  u
is embarrassingly parallel, and any self-consistent trajectory is (by
induction over t) THE unique reference trajectory. On verification failure
the slow-but-general sequential module (previous baseline, kept verbatim
below) is built and used instead.
"""

import numpy as np

B, L, V, D = 1024, 1024, 20, 128
NCORES = 8
BLOC = B // NCORES  # 128 batch rows per core
J2 = 2 * V          # 40

_CACHE = {}

# ---------------------------------------------------------------------------
# fast streaming module
# ---------------------------------------------------------------------------

# consts blob [128, FCW] column layout
_FOFF = {'wrow': 0, 'iota20m': 20, 'rr0': 40, 'pp': 41, 'ppfk': 42, 'cadd': 43}
FCW = 44

# floor((tp2)/20) via round-to-nearest: q1 = RN(tp2*FK + FC) = floor(tp2/20)+1
# for integer tp2 in [0, 741]; validated exhaustively (FC in [0.505, 0.547]).
FK = 3277.0 / 65536.0
FC = 0.525
FM = 8388608.0  # 2^23

TCH = 80                    # steps per chunk (DMA/compute overlap sweet spot)
TPATCH = 3                  # host-patched leading steps
XM_BF16 = True              # bf16 intermediate for the weighted-max pass
MULT_ENGINE = "gpsimd"      # engine for the x*w pass: "gpsimd" | "vector"


def _inv_table():
    inv = np.zeros(V, dtype=np.int64)
    for k in range(1, V):
        if np.gcd(k, V) == 1:
            inv[k] = pow(k, -1, V)
    return inv


def _chunk_schedule(nsteps, tch, ramp, tail=0):
    """Chunk sizes: optionally smaller leading/trailing chunks to shorten the
    pipeline fill/drain, full-width chunks in the middle. `tail` may be an
    int or a tuple of trailing chunk sizes (largest first)."""
    sizes = []
    left = nsteps
    if ramp and nsteps > tch:
        for r in ((ramp,) if isinstance(ramp, int) else tuple(ramp)):
            r = min(r, left)
            if r > 0:
                sizes.append(r)
                left -= r
    tails = []
    if nsteps > tch:
        for t in ((tail,) if isinstance(tail, int) else tuple(tail)):
            t = min(t, left)
            if t > 0:
                tails.append(t)
                left -= t
    while left > 0:
        s = min(tch, left)
        sizes.append(s)
        left -= s
    sizes.extend(tails)
    return sizes


def _build_fast_module(nsteps, tch=None, xm_bf16=None, mult_engine=None,
                       xbufs=6, wbufs=4, obufs=6, ramp=40, tail=(40, 16, 8),
                       barrier=False, dve_tail_n=0):
    import concourse.bacc as bacc
    import concourse.mybir as mybir
    import concourse.tile as tile

    if tch is None:
        tch = TCH
    if xm_bf16 is None:
        xm_bf16 = XM_BF16
    if mult_engine is None:
        mult_engine = MULT_ENGINE
    f32 = mybir.dt.float32
    xm_dt = mybir.dt.bfloat16 if xm_bf16 else f32
    nc = bacc.Bacc()

    x_d = nc.declare_dram_parameter("xloc", [BLOC, nsteps, V], f32, isOutput=False)
    consts_d = nc.declare_dram_parameter("consts", [128, FCW], f32, isOutput=False)
    ys_d = nc.declare_dram_parameter("ys", [BLOC, nsteps, V], f32, isOutput=True)

    tch = min(tch, nsteps)
    sched = _chunk_schedule(nsteps, tch, ramp, tail)

    sub = mybir.AluOpType.subtract
    mult = mybir.AluOpType.mult
    add = mybir.AluOpType.add
    is_eq = mybir.AluOpType.is_equal
    axX = mybir.AxisListType.X

    with tile.TileContext(nc) as tc:
        with (
            tc.tile_pool(name="persist", bufs=1) as pp,
            tc.tile_pool(name="xstage", bufs=xbufs) as xp,
            tc.tile_pool(name="work", bufs=wbufs) as wp,
            tc.tile_pool(name="ostage", bufs=obufs) as op,
        ):
            cblob = pp.tile([128, FCW], f32, tag="cblob")
            # consts go through the gpsimd SWDGE path: neither the SP queue
            # (first x-chunk DMA) nor the HWDGE generator is delayed by it
            nc.gpsimd.dma_start(out=cblob[:], in_=consts_d[:])
            o = _FOFF
            wrow = cblob[:, o['wrow']:o['wrow'] + V]
            iota20m = cblob[:, o['iota20m']:o['iota20m'] + V]
            rr0 = cblob[:, o['rr0']:o['rr0'] + 1]
            ppc = cblob[:, o['pp']:o['pp'] + 1]
            ppfk = cblob[:, o['ppfk']:o['ppfk'] + 1]
            cadd = cblob[:, o['cadd']:o['cadd'] + 1]
            if barrier:
                tc.strict_bb_all_engine_barrier()

            t0 = 0
            for ci, ncols in enumerate(sched):
                xt = xp.tile([BLOC, tch, V], f32, tag="xt")
                nc.sync.dma_start(out=xt[:, :ncols, :],
                                  in_=x_d[:, t0:t0 + ncols, :])
                # xm = x * (40 - v): one nonzero (= 40 - a) per (b, t) window
                # (trailing drain chunks may run the mult on DVE to drop the
                # gpsimd handoff from the critical drain path)
                on_dve = (mult_engine != "gpsimd"
                          or ci >= len(sched) - dve_tail_n)
                xm = wp.tile([BLOC, tch, V], xm_dt, tag="xm")
                meng = nc.vector if on_dve else nc.gpsimd
                meng.tensor_tensor(
                    out=xm[:, :ncols, :], in0=xt[:, :ncols, :],
                    in1=wrow[:].unsqueeze(1).broadcast_to((BLOC, ncols, V)),
                    op=mult)
                # a5 = 40 - a  (window max; exact, entries are 0 or 40-a >= 21)
                a5 = wp.tile([BLOC, tch], xm_dt, tag="a5")
                nc.vector.reduce_max(out=a5[:, :ncols], in_=xm[:, :ncols, :],
                                     axis=axX)
                # tp2 = (a5 - (20 - m*)) * p*   in [0, 741]
                tp2 = wp.tile([BLOC, tch], f32, tag="tp2")
                nc.vector.tensor_scalar(
                    out=tp2[:, :ncols], in0=a5[:, :ncols], scalar1=rr0,
                    op0=sub, scalar2=ppc, op1=mult)
                # yf = a5*(p*FK) + (FC - rr0*p*FK) = tp2*FK + FC  (one rounding
                # more than via tp2; error <= 3e-6 vs the 3e-3 floor margin).
                # q1 = RN(yf) = floor(tp2/20) + 1  (exact)
                yf = wp.tile([BLOC, tch], f32, tag="yf")
                nc.vector.tensor_scalar(
                    out=yf[:, :ncols], in0=a5[:, :ncols], scalar1=ppfk,
                    op0=mult, scalar2=cadd, op1=add)
                qf = wp.tile([BLOC, tch], f32, tag="qf")
                nc.vector.tensor_scalar(
                    out=qf[:, :ncols], in0=yf[:, :ncols], scalar1=FM,
                    op0=add, scalar2=FM, op1=sub)
                # um = tp2 - 20*q1 = (tp2 mod 20) - 20   in [-20, -1]
                um = wp.tile([BLOC, tch], f32, tag="um")
                nc.vector.scalar_tensor_tensor(
                    out=um[:, :ncols], in0=qf[:, :ncols], scalar=-20.0,
                    op0=mult, in1=tp2[:, :ncols], op1=add)
                # oh[b,t,v] = (um == v - 20)
                oh = op.tile([BLOC, tch, V], f32, tag="oh")
                nc.vector.tensor_tensor(
                    out=oh[:, :ncols, :],
                    in0=um[:, :ncols].unsqueeze(2).broadcast_to(
                        (BLOC, ncols, V)),
                    in1=iota20m[:].unsqueeze(1).broadcast_to((BLOC, ncols, V)),
                    op=is_eq)
                # the drain-gating final flush goes out on the SP queue,
                # which is idle by then (all x-chunk DMAs long issued)
                oeng = nc.sync if ci == len(sched) - 1 else nc.scalar
                oeng.dma_start(out=ys_d[:, t0:t0 + ncols, :],
                               in_=oh[:, :ncols, :])
                t0 += ncols

    nc.finalize()
    return nc


def _build_fast_consts(b, EW, a0):
    """Bootstrap 3 steps on the count formulation; returns per-row frozen
    scalars and the first-3-step symbols for the host patch."""
    f32 = np.float32
    inv = _inv_table()
    batch = a0.shape[0]
    net = np.broadcast_to(b.astype(f32), (batch, J2)).copy()   # net_0 = b
    us = []
    for t in range(TPATCH):
        m = np.argmax(net[:, :V], axis=-1)
        s = np.argmax(net[:, V:], axis=-1)
        u = ((a0[:, t] - m) * inv[s]) % V
        us.append(u)
        net = net + EW[u]
    m = np.argmax(net[:, :V], axis=-1)          # frozen (m*, s*)
    s = np.argmax(net[:, V:], axis=-1)
    pstar = (V - inv[s]) % V
    rr0 = (V - m).astype(f32)                   # 20 - m*
    pp = pstar.astype(f32)
    return rr0, pp, np.stack(us, axis=1)        # [B], [B], [B, TPATCH]


def _make_fast_in_maps(x, rr0, pp, nsteps):
    f32 = np.float32
    in_maps = []
    base = np.zeros((128, FCW), dtype=f32)
    o = _FOFF
    base[:, o['wrow']:o['wrow'] + V] = (40.0 - np.arange(V, dtype=f32))[None, :]
    base[:, o['iota20m']:o['iota20m'] + V] = (np.arange(V, dtype=f32)
                                              - 20.0)[None, :]
    for c in range(NCORES):
        blob = base.copy()
        r = rr0[c * BLOC:(c + 1) * BLOC].astype(np.float64)
        p = pp[c * BLOC:(c + 1) * BLOC].astype(np.float64)
        blob[:, o['rr0']] = rr0[c * BLOC:(c + 1) * BLOC]
        blob[:, o['pp']] = pp[c * BLOC:(c + 1) * BLOC]
        blob[:, o['ppfk']] = (p * FK).astype(f32)
        blob[:, o['cadd']] = (FC - r * (p * FK).astype(f32).astype(np.float64)
                              ).astype(f32)
        xl = np.ascontiguousarray(x[c * BLOC:(c + 1) * BLOC, :nsteps, :], f32)
        in_maps.append(dict(xloc=xl, consts=blob))
    return in_maps


def _verify_fast(out, a, b, EW):
    """Vectorized fixed-point check: the output trajectory is self-consistent
    under the reference recurrence (sufficient: it IS the reference output,
    by induction over t). Non-coprime s is covered by the same formula
    (INV_P maps it to index 0, i.e. one_hot(0), which inv[s]=0 reproduces)."""
    inv = _inv_table()
    batch, nsteps = out.shape[0], out.shape[1]
    if np.count_nonzero(out) != batch * nsteps:
        return False
    if float(np.sum(out)) != float(batch * nsteps):   # all nonzeros exactly 1.0
        return False
    u_dev = np.argmax(out, axis=-1)                       # [B, L]
    rowE = out.reshape(-1, V) @ EW                        # EW[u_t] rows (BLAS)
    rowE = rowE.reshape(batch, nsteps, J2)
    net = np.cumsum(rowE, axis=1) - rowE + b.astype(np.float32)
    m = np.argmax(net[..., :V], axis=-1)
    s = np.argmax(net[..., V:], axis=-1)
    u_chk = ((a - m) * inv[s]) % V
    return bool(np.array_equal(u_chk, u_dev))


# ---------------------------------------------------------------------------
# sequential fallback module (previous baseline, verbatim)
# ---------------------------------------------------------------------------

# column offsets inside the single consts blob [128, CONSTS_W]
_COFF = {'emb32': 0, 'wout': 128, 'brow128': 168, 'cmab': 208, 'c2': 248,
         'iota32': 268, 'c20j': 300, 'ewb32': 320, 'ident': 360}
CONSTS_W = 488


def _build_consts_blob(emb, W_out, b):
    """Host-side constants packed into one [128, CONSTS_W] fp32 blob."""
    f32 = np.float32
    blob = np.zeros((128, CONSTS_W), dtype=f32)
    o = _COFF
    blob[:V, o['emb32']:o['emb32'] + D] = emb
    blob[:, o['wout']:o['wout'] + J2] = W_out
    blob[:, o['brow128']:o['brow128'] + J2] = b[None, :]
    ewb = (emb.astype(np.float64) @ W_out.astype(np.float64)
           + b.astype(np.float64)).astype(f32)
    blob[:V, o['ewb32']:o['ewb32'] + J2] = ewb
    blob[:, o['ident']:o['ident'] + 128] = np.eye(128, dtype=f32)
    inv = _inv_table()
    p = (V - inv) % V
    j = np.arange(V)
    c0 = (V - j).astype(f32)
    blob[:, o['cmab']:o['cmab'] + J2] = np.concatenate([c0, c0])[None, :]
    blob[:, o['c2']:o['c2'] + V] = (c0 + p.astype(f32) / 64.0)[None, :]
    iota32 = np.arange(32, dtype=f32) - 380.0
    iota32[V:] = 1000.0
    blob[:, o['iota32']:o['iota32'] + 32] = iota32[None, :]
    blob[:, o['c20j']:o['c20j'] + V] = (V - np.arange(V, dtype=f32))[None, :]
    return blob


def _build_seq_module(nsteps):
    import concourse.bacc as bacc
    import concourse.mybir as mybir
    import concourse.tile as tile
    from concourse.tile_rust import add_dep_helper

    f32 = mybir.dt.float32
    nc = bacc.Bacc()

    x_d = nc.declare_dram_parameter("xloc", [BLOC, nsteps, V], f32, isOutput=False)
    consts_d = nc.declare_dram_parameter("consts", [128, CONSTS_W], f32,
                                         isOutput=False)
    ys_d = nc.declare_dram_parameter("ys", [BLOC, nsteps, V], f32, isOutput=True)

    XCH = min(128, nsteps)
    nxch = (nsteps + XCH - 1) // XCH
    YCH = min(64, nsteps)
    RING = 2 * YCH

    sub = mybir.AluOpType.subtract
    mult = mybir.AluOpType.mult
    add = mybir.AluOpType.add
    is_ge = mybir.AluOpType.is_ge
    is_eq = mybir.AluOpType.is_equal
    axX = mybir.AxisListType.X

    with tile.TileContext(nc) as tc:
        with (
            tc.tile_pool(name="persist", bufs=1) as pp,
            tc.tile_pool(name="xstage", bufs=2) as xp,
            tc.tile_pool(name="psum", bufs=1, space="PSUM") as pspool,
        ):
            cblob = pp.tile([128, CONSTS_W], f32, tag="cblob")
            nc.sync.dma_start(out=cblob[:], in_=consts_d[:])
            o = _COFF
            emb32 = cblob[0:32, o['emb32']:o['emb32'] + D]
            wout = cblob[:, o['wout']:o['wout'] + J2]
            brow128 = cblob[:, o['brow128']:o['brow128'] + J2]
            cmab = cblob[:, o['cmab']:o['cmab'] + J2]
            c2 = cblob[:, o['c2']:o['c2'] + V]
            iota32 = cblob[:, o['iota32']:o['iota32'] + 32]
            c20j = cblob[:, o['c20j']:o['c20j'] + V]
            ewb32 = cblob[0:32, o['ewb32']:o['ewb32'] + J2]
            ident = cblob[:, o['ident']:o['ident'] + 128]
            tc.strict_bb_all_engine_barrier()

            a5 = pp.tile([BLOC, nsteps], f32, tag="a5")
            ysring = pp.tile([BLOC, RING, 32], f32, tag="ysring")
            stateT = pp.tile([D, BLOC], f32, tag="stateT")
            dfm = pp.tile([BLOC, J2], f32, tag="dfm")
            msk = pp.tile([BLOC, 3 * V], f32, tag="msk")
            mx = pp.tile([BLOC, 2], f32, tag="mx")
            rr3 = pp.tile([BLOC, 3], f32, tag="rr3")
            pf = pp.tile([BLOC, 1], f32, tag="pf")
            tpf = pp.tile([BLOC, 1], f32, tag="tpf")
            yf = pp.tile([BLOC, 1], f32, tag="yf")
            qf = pp.tile([BLOC, 1], f32, tag="qf")
            q20f = pp.tile([BLOC, 1], f32, tag="q20f")
            ohT = pp.tile([32, BLOC], f32, tag="ohT")

            net_ps = [pspool.tile([BLOC, 512], f32, tag=f"net_ps{i}",
                                  name=f"net_ps{i}")[:, 0:J2]
                      for i in range(2)]
            ohT_ps = pspool.tile([32, BLOC], f32, tag="ohT_ps")
            state_ps = pspool.tile([D, BLOC], f32, tag="state_ps")

            nc.gpsimd.memset(stateT[:], 0.0)
            nc.vector.tensor_copy(out=net_ps[0][:], in_=brow128[:])

            for c in range(nxch):
                t0 = c * XCH
                ncols = min(XCH, nsteps - t0)
                xt = xp.tile([BLOC, XCH, V], f32, tag="xt")
                xm = xp.tile([BLOC, XCH, V], f32, tag="xm")
                nc.sync.dma_start(out=xt[:, :ncols, :], in_=x_d[:, t0:t0 + ncols, :])
                nc.vector.tensor_tensor(
                    out=xm[:, :ncols, :], in0=xt[:, :ncols, :],
                    in1=c20j[:].unsqueeze(1).broadcast_to((BLOC, ncols, V)),
                    op=mult)
                nc.vector.reduce_sum(out=a5[:, t0:t0 + ncols], in_=xm[:, :ncols, :],
                                     axis=axX)

            for t in range(nsteps):
                nA = net_ps[t % 2]
                nB = net_ps[(t + 1) % 2]

                nc.vector.reduce_max(out=mx[:], in_=nA[:].rearrange(
                    "b (h v) -> b h v", h=2), axis=axX)
                nc.vector.tensor_tensor(
                    out=dfm[:].rearrange("b (h v) -> b h v", h=2),
                    in0=nA[:].rearrange("b (h v) -> b h v", h=2),
                    in1=mx[:].unsqueeze(2).broadcast_to((BLOC, 2, V)),
                    op=sub)
                nc.vector.scalar_tensor_tensor(
                    out=msk[:, 0:J2], in0=dfm[:], scalar=0.0, in1=cmab[:],
                    op0=is_ge, op1=mult)
                nc.vector.scalar_tensor_tensor(
                    out=msk[:, J2:3 * V], in0=dfm[:, V:J2], scalar=0.0,
                    in1=c2[:], op0=is_ge, op1=mult)
                nc.vector.reduce_max(out=rr3[:], in_=msk[:].rearrange(
                    "b (h v) -> b h v", h=3), axis=axX)
                nc.vector.tensor_scalar(
                    out=pf[:], in0=rr3[:, 2:3], scalar1=rr3[:, 1:2], op0=sub,
                    scalar2=64.0, op1=mult)
                nc.vector.tensor_scalar(
                    out=tpf[:], in0=a5[:, t:t + 1], scalar1=rr3[:, 0:1],
                    op0=sub, scalar2=pf[:], op1=mult)
                nc.vector.tensor_scalar(
                    out=yf[:], in0=tpf[:], scalar1=0.0500030517578125,
                    op0=mult, scalar2=18.50025177001953125, op1=add)
                nc.vector.tensor_scalar(
                    out=qf[:], in0=yf[:], scalar1=8388608.0, op0=add,
                    scalar2=8388608.0, op1=sub)
                nc.vector.tensor_scalar(
                    out=q20f[:], in0=qf[:], scalar1=20.0, scalar2=None,
                    op0=mult)
                ohs = ysring[:, t % RING, :]
                nc.vector.tensor_scalar(
                    out=ohs, in0=iota32[:], scalar1=q20f[:], op0=add,
                    scalar2=tpf[:], op1=is_eq)
                if t > 0:
                    nc.scalar.copy(out=stateT[:], in_=state_ps[:])

                mm1 = nc.tensor.matmul(nB[:], stateT[:], wout[:],
                                       start=True, stop=False,
                                       skip_group_check=True)

                nc.tensor.transpose(ohT_ps[:], ohs, ident[:])
                nc.vector.tensor_copy(out=ohT[:], in_=ohT_ps[:])

                m4 = nc.tensor.matmul(nB[:], ohT[:], ewb32[:],
                                      start=False, stop=True,
                                      skip_group_check=True)
                add_dep_helper(m4.ins, mm1.ins, sync=False,
                               reason="net accum order")
                nc.tensor.matmul(state_ps[:], emb32[:], ohT[:],
                                 start=(t == 0), stop=(t == nsteps - 1),
                                 skip_group_check=True)

                if (t + 1) % YCH == 0:
                    h0 = (t + 1 - YCH) % RING
                    nc.sync.dma_start(
                        out=ys_d[:, t + 1 - YCH:t + 1, :],
                        in_=ysring[:, h0:h0 + YCH, :V])
            if nsteps % YCH:
                tdone = (nsteps // YCH) * YCH
                h0 = tdone % RING
                nc.sync.dma_start(
                    out=ys_d[:, tdone:nsteps, :],
                    in_=ysring[:, h0:h0 + (nsteps - tdone), :V])

    nc.finalize()
    return nc


def _make_seq_in_maps(x, emb, W_out, b, nsteps):
    f32 = np.float32
    blob = _build_consts_blob(np.asarray(emb, f32), np.asarray(W_out, f32),
                              np.asarray(b, f32))
    in_maps = []
    for c in range(NCORES):
        xl = np.ascontiguousarray(x[c * BLOC:(c + 1) * BLOC, :nsteps, :], f32)
        in_maps.append(dict(xloc=xl, consts=blob))
    return in_maps


# ---------------------------------------------------------------------------
# host driver
# ---------------------------------------------------------------------------

def _get_module(nsteps):
    key = ("fast", nsteps)
    if key not in _CACHE:
        _CACHE[key] = _build_fast_module(nsteps)
    return _CACHE[key]


def _get_seq_module(nsteps):
    key = ("seq", nsteps)
    if key not in _CACHE:
        _CACHE[key] = _build_seq_module(nsteps)
    return _CACHE[key]


def _run_spmd(nc, in_maps):
    from concourse.bass_utils import run_bass_kernel_spmd
    # the axon-tunneled device occasionally reports a transient
    # "unrecoverable" execution failure; one retry has been observed to
    # succeed
    last_err = None
    for _ in range(3):
        try:
            return run_bass_kernel_spmd(nc, in_maps, list(range(NCORES)))
        except Exception as e:  # jax.errors.JaxRuntimeError and friends
            last_err = e
    raise last_err


def _make_in_maps(x, emb, W_out, b, nsteps):
    """Fast-path in_maps (kept for test.py compatibility)."""
    f32 = np.float32
    EW = (np.asarray(emb, np.float64) @ np.asarray(W_out, np.float64)).astype(f32)
    a = np.argmax(np.asarray(x, f32), axis=-1)
    rr0, pp, _ = _build_fast_consts(np.asarray(b, f32), EW, a)
    return _make_fast_in_maps(np.asarray(x, f32), rr0, pp, nsteps)


def kernel(x, emb, W_out, b):
    f32 = np.float32
    x = np.asarray(x, f32)
    emb = np.asarray(emb, f32)
    W_out = np.asarray(W_out, f32)
    b = np.asarray(b, f32)

    nsteps = x.shape[1]
    EW = (emb.astype(np.float64) @ W_out.astype(np.float64)).astype(f32)
    a = np.argmax(x, axis=-1)                       # [B, L] symbol indices

    rr0, pp, us_early = _build_fast_consts(b, EW, a)
    nc = _get_module(nsteps)
    in_maps = _make_fast_in_maps(x, rr0, pp, nsteps)
    res = _run_spmd(nc, in_maps)
    out = np.concatenate([res.results[c]["ys"] for c in range(NCORES)], axis=0)
    # patch the pre-freeze steps computed on host
    npatch = min(TPATCH, nsteps)
    eye = np.eye(V, dtype=f32)
    out[:, :npatch, :] = eye[us_early[:, :npatch]]

    if _verify_fast(out, a, b, EW):
        return out.astype(f32)

    # attractor assumption failed for these inputs: run the general
    # sequential module instead
    nc = _get_seq_module(nsteps)
    in_maps = _make_seq_in_maps(x, emb, W_out, b, nsteps)
    res = _run_spmd(nc, in_maps)
    out = np.concatenate([res.results[c]["ys"] for c in range(NCORES)], axis=0)
    return out.astype(f32)
